# revision 2
# baseline (speedup 1.0000x reference)
"""Causal multi-head self-attention (B=2, S=4096, D=1024, H=16, dk=64) on 8 trn2 cores.

Sharding: core c handles batch b = c // 4 and heads [4*(c%4) .. 4*(c%4)+3]
(data parallel on B, tensor parallel on heads / QKV / O projections).
Each core returns a partial [S, D] output (its heads' contribution after the
Wo projection); the host sums the 4 partials per batch.

Device-side design (per core):
  - host supplies xT = x[b].T so every projection contracts over the model dim
    on partitions; Wq/Wk columns are host-permuted so RoPE is rotate-half form
    (full-width DVE ops), then SBUF->SBUF DMAs re-group rotated rows
    head-contiguously (bf16) for the K=64 QK^T contraction.
  - attention runs in S^T layout (scores [k, q]). V carries 64 ones-columns
    per head, so each PV matmul emits O^T on partitions 0:64 AND the softmax
    denominator l broadcast across partitions 64:128 - normalization is just
    copy + reciprocal + multiply, no partition-axis reduction or shuffle.
  - K/Q/V/probs/Wo run in bf16 (same PE rate, half SBUF, 2x DVE on masks);
    projections contract in float32r, PSUM accumulates fp32 throughout.
  - instruction-level software pipeline: attention for q-tile t-1 is the
    backbone (QK staging runs 2 key-block groups ahead of exp -> mask -> PV),
    and the projection / Wo-output matmul chunks of neighboring tiles are
    spread between groups so the PE never drains while ACT computes exp.
"""

import numpy as np
import ml_dtypes

import concourse.bass as bass
import concourse.bacc as bacc
import concourse.mybir as mybir
import concourse.tile as tile
from concourse.bass_utils import run_bass_kernel_spmd

P = 128
D_MODEL = 1024
N_HEADS = 16
D_K = 64
SEQ = 4096
BATCH = 2
N_CORES = 8
HPC = 4  # heads per core
ST = 512  # s-tile / q-tile width
GK = 2  # key-blocks per staging / exp group
THETA = 10000.0

f32 = mybir.dt.float32
f32r = mybir.dt.float32r
bf16 = mybir.dt.bfloat16
AF = mybir.ActivationFunctionType
OP = mybir.AluOpType


def build_program(S=SEQ, reps=1, debug_taps=False):
    nc = bacc.Bacc("TRN2", target_bir_lowering=False, debug=False,
                   num_devices=N_CORES)

    NT = S // ST          # number of s-tiles == number of q-tiles
    NKB = S // P          # number of 128-wide key blocks
    NIC = D_MODEL // P    # contraction chunks over the model dim

    xT_d = nc.dram_tensor("xT", [D_MODEL, S], f32r, kind="ExternalInput").ap()
    wqkv_d = nc.dram_tensor("wqkvT", [D_MODEL, 768], f32r, kind="ExternalInput").ap()
    wo_d = nc.dram_tensor("woT", [256, D_MODEL], bf16, kind="ExternalInput").ap()
    cos_d = nc.dram_tensor("cos4", [P, S], f32, kind="ExternalInput").ap()
    sin_d = nc.dram_tensor("sin4", [P, S], f32, kind="ExternalInput").ap()
    mask_d = nc.dram_tensor("dmask", [P, 4 * ST], bf16, kind="ExternalInput").ap()
    y_d = nc.dram_tensor("y", [S, D_MODEL], f32, kind="ExternalOutput").ap()
    if debug_taps:
        dbg_es = nc.dram_tensor("dbg_es", [P, GK, ST], bf16,
                                kind="ExternalOutput").ap()
        dbg_of = nc.dram_tensor("dbg_of", [P, ST], f32,
                                kind="ExternalOutput").ap()
        dbg_rl = nc.dram_tensor("dbg_rl", [64, ST], f32,
                                kind="ExternalOutput").ap()
        dbg_qt = nc.dram_tensor("dbg_qt", [P, 2, ST], bf16,
                                kind="ExternalOutput").ap()
        dbg_kt = nc.dram_tensor("dbg_kt", [P, 2, ST], bf16,
                                kind="ExternalOutput").ap()
        dbg_v = nc.dram_tensor("dbg_v", [P, HPC, 128], bf16,
                               kind="ExternalOutput").ap()

    with tile.TileContext(nc) as tc:
      for _rep in range(reps):
        with tc.tile_pool(name="res", bufs=1) as res:
            KT = res.tile([P, 2, S], bf16)       # rotated K^T, head-contiguous
            V = res.tile([P, NKB, HPC, 128], bf16)  # 64 val + 64 ones cols/head
            nc.gpsimd.memset(V[:, :, :, 64:128], 1.0)

            with tc.tile_pool(name="p1", bufs=2) as p1, \
                 tc.tile_pool(name="rot", bufs=2) as rotp, \
                 tc.tile_pool(name="p2", bufs=2) as p2, \
                 tc.tile_pool(name="ppp", bufs=2, space="PSUM") as ppp, \
                 tc.tile_pool(name="stg", bufs=2, space="PSUM") as stgp, \
                 tc.tile_pool(name="opp", bufs=2, space="PSUM") as opp:
                xT3 = xT_d.rearrange("(a p) s -> p a s", p=P)
                xt_tiles, cs_tiles, qt_tiles, ot_tiles = {}, {}, {}, {}

                def dma_in(t):
                    # per-chunk DMAs so the first projection matmul only
                    # waits on chunk 0, not the whole 2MB tile
                    tsl = slice(t * ST, (t + 1) * ST)
                    xt = p1.tile([P, NIC, ST], f32r, tag="xt", bufs=2)
                    for ic in range(NIC):
                        nc.sync.dma_start(xt[:, ic, :], xT3[:, ic, tsl])
                    cs = p1.tile([P, 2, ST], f32, tag="cs", bufs=2)
                    nc.sync.dma_start(cs[:, 0, :], cos_d[:, tsl])
                    nc.sync.dma_start(cs[:, 1, :], sin_d[:, tsl])
                    xt_tiles[t] = xt
                    cs_tiles[t] = cs

                def make_proj_units(t):
                    """PE/DVE/DMA work units for projecting s-tile t."""
                    tsl = slice(t * ST, (t + 1) * ST)
                    state = {}

                    def mm_qk(col0, key, pool, ptag):
                        def u():
                            xt = xt_tiles[t]
                            pe_ps = pool.tile([P, ST], f32, tag=ptag)
                            po_ps = pool.tile([P, ST], f32, tag=ptag)
                            for ic in range(NIC):
                                nc.tensor.matmul(
                                    pe_ps,
                                    lhsT=wqkv_sb[:, ic, col0:col0 + P],
                                    rhs=xt[:, ic, :],
                                    start=(ic == 0), stop=(ic == NIC - 1))
                                nc.tensor.matmul(
                                    po_ps,
                                    lhsT=wqkv_sb[:, ic, col0 + P:col0 + 256],
                                    rhs=xt[:, ic, :],
                                    start=(ic == 0), stop=(ic == NIC - 1))
                            state[key] = (pe_ps, po_ps)
                        return u

                    def rope(key, is_q):
                        def u():
                            pe_ps, po_ps = state.pop(key)
                            cs = cs_tiles[t]
                            if is_q:
                                dst = p2.tile([P, 2, ST], bf16, tag="qt",
                                              bufs=2)
                                qt_tiles[t] = dst
                                dsl = slice(0, ST)
                            else:
                                dst = KT
                                dsl = tsl
                            t1 = rotp.tile([P, ST], f32, tag="tmp", bufs=5)
                            t3 = rotp.tile([P, ST], f32, tag="tmp", bufs=5)
                            nc.vector.tensor_tensor(t1, pe_ps, cs[:, 0, :], OP.mult)
                            nc.vector.tensor_tensor(t3, pe_ps, cs[:, 1, :], OP.mult)
                            t2 = rotp.tile([P, ST], f32, tag="tmp", bufs=5)
                            t4 = rotp.tile([P, ST], f32, tag="tmp", bufs=5)
                            nc.vector.tensor_tensor(t2, po_ps, cs[:, 1, :], OP.mult)
                            nc.vector.tensor_tensor(t4, po_ps, cs[:, 0, :], OP.mult)
                            rot_e = rotp.tile([P, ST], bf16, tag="re")
                            rot_o = rotp.tile([P, ST], bf16, tag="ro")
                            nc.vector.tensor_tensor(rot_e, t1, t2, OP.subtract)
                            nc.vector.tensor_tensor(rot_o, t3, t4, OP.add)
                            # scatter rotated rows head-contiguously:
                            # head h evens -> dst[(h%2)*64 +  0 .. +32, h//2]
                            # head h odds  -> dst[(h%2)*64 + 32 .. +64, h//2]
                            for j in (0, 1):
                                for hh in (0, 1):
                                    h = 2 * j + hh
                                    nc.sync.dma_start(
                                        dst[hh * 64:hh * 64 + 32, j, dsl],
                                        rot_e[h * 32:(h + 1) * 32, :])
                                    nc.sync.dma_start(
                                        dst[hh * 64 + 32:hh * 64 + 64, j, dsl],
                                        rot_o[h * 32:(h + 1) * 32, :])
                        return u

                    def vproj(sb0):
                        def u():
                            xt = xt_tiles[t]
                            for sb in (sb0, sb0 + 1):
                                kb = t * (ST // P) + sb
                                v_ps = ppp.tile([P, 256], f32, tag="pp")
                                for ic in range(NIC):
                                    nc.tensor.matmul(
                                        v_ps,
                                        lhsT=xt[:, ic, sb * P:(sb + 1) * P],
                                        rhs=wqkv_sb[:, ic, 512:768],
                                        start=(ic == 0), stop=(ic == NIC - 1))
                                nc.vector.tensor_copy(
                                    V[:, kb, :, 0:64],
                                    v_ps.rearrange("p (h c) -> p h c", h=HPC))
                        return u

                    units = []
                    if t + 1 < NT:
                        units.append(lambda: dma_in(t + 1))
                    # tile 0 runs before any attention exists to hide PSUM
                    # write-after-read stalls; spread it over the idle
                    # attention pools for extra bank parallelism
                    if t == 0:
                        units.append(mm_qk(0, "q", ppp, "pp"))
                        units.append(mm_qk(256, "k", opp, "o"))
                        units.append(rope("q", True))
                        units.append(rope("k", False))
                    else:
                        units.append(mm_qk(0, "q", ppp, "pp"))
                        units.append(rope("q", True))
                        units.append(mm_qk(256, "k", ppp, "pp"))
                        units.append(rope("k", False))
                    units.append(vproj(0))
                    units.append(vproj(2))
                    return units

                def make_wo_units(t):
                    """Wo projection for q-tile t's 4 output row-blocks."""
                    ot = ot_tiles.pop(t)
                    state = {}

                    def u_make(sb, nh):
                        def u():
                            if nh == 0:
                                state[sb] = p2.tile([P, D_MODEL], f32,
                                                    tag="y", bufs=2,
                                                    name=f"out_t{t}_{sb}")
                            out_t = state[sb]
                            y_ps = ppp.tile([P, 512], f32, tag="pp")
                            for j in (0, 1):
                                nc.tensor.matmul(
                                    y_ps,
                                    lhsT=ot[:, j, sb * P:(sb + 1) * P],
                                    rhs=wo_sb[:, j, nh * 512:(nh + 1) * 512],
                                    start=(j == 0), stop=(j == 1))
                            nc.vector.tensor_copy(
                                out_t[:, nh * 512:(nh + 1) * 512], y_ps)
                            if nh == 1:
                                nc.sync.dma_start(
                                    y_d[t * ST + sb * P:t * ST + (sb + 1) * P, :],
                                    out_t)
                        return u

                    return [u_make(sb, nh) for sb in range(ST // P)
                            for nh in (0, 1)]

                def emit_attention(qt, units):
                    """Attention for q-tile qt with a 2-group run-ahead;
                    `units` (proj/wo chunks) are spread between groups."""
                    nkb = (qt + 1) * (ST // P)
                    qt_tile = qt_tiles.pop(qt)
                    ot_acc = p2.tile([P, 2, ST], bf16, tag="ota", bufs=2)
                    ot_tiles[qt] = ot_acc

                    # head pairs (even, odd) are interleaved group-by-group:
                    # even heads sit at PE base partition 0, odd heads at 64,
                    # so adjacent K=64 matmuls land on disjoint row-groups of
                    # the PE array and run concurrently (~2x QK throughput)
                    glist = []  # (h, g0, glen, q0s, is_last_of_head)
                    for hp in range(HPC // 2):
                        for g0 in range(0, nkb, GK):
                            glen = min(GK, nkb - g0)
                            q0s = 256 if g0 - qt * (ST // P) >= 2 else 0
                            last = g0 + GK >= nkb
                            for h in (2 * hp, 2 * hp + 1):
                                glist.append((h, g0, glen, q0s, last))
                    stgs = [None] * len(glist)
                    o_ps_by_head = {}

                    def emit_qk(i):
                        h, g0, glen, q0s, _ = glist[i]
                        j, hb = h // 2, (h % 2) * 64
                        stg = stgp.tile([P, GK, ST], f32, tag="s")
                        for gi in range(glen):
                            kb = g0 + gi
                            # per-block trim: cols < 128*c are fully masked
                            # and never consumed downstream (exp output in
                            # [q0s:q0k) is discarded garbage)
                            c = kb - qt * (ST // P)
                            q0k = max(c, 0) * P
                            nc.tensor.matmul(
                                stg[:, gi, q0k:],
                                lhsT=KT[hb:hb + 64, j, kb * P:(kb + 1) * P],
                                rhs=qt_tile[hb:hb + 64, j, q0k:],
                                start=True, stop=True)
                        stgs[i] = stg

                    def emit_tail(i):
                        h, g0, glen, q0s, last = glist[i]
                        j, hb = h // 2, (h % 2) * 64
                        stg = stgs[i]
                        stgs[i] = None
                        es = p2.tile([P, GK, ST], bf16, tag="e", bufs=3)
                        nc.scalar.activation(es[:, :glen, q0s:],
                                             stg[:, :glen, q0s:], AF.Exp)
                        if g0 == 0:
                            o_ps_by_head[h] = opp.tile(
                                [P, ST], f32, tag="o", name=f"o_ps_h{h}")
                        o_ps = o_ps_by_head[h]
                        for gi in range(glen):
                            kb = g0 + gi
                            c = kb - qt * (ST // P)
                            q0p = max(c, 0) * P
                            if c >= 0:
                                # zero exp(S^T)[i, jq] where jq < i + 128*c
                                nc.vector.tensor_tensor(
                                    es[:, gi, q0p:], es[:, gi, q0p:],
                                    mask_sb[:, c, q0p:], OP.mult)
                            nc.tensor.matmul(
                                o_ps[:, q0p:],
                                lhsT=V[:, kb, h, :],
                                rhs=es[:, gi, q0p:],
                                start=(kb == 0), stop=(kb == nkb - 1))
                        if debug_taps and qt == 0 and h == 0 and g0 == 0:
                            nc.sync.dma_start(dbg_es, es)
                        if last:
                            # rows 64:128 of o_ps hold l broadcast by the
                            # ones-columns of V; normalize into the bf16 O^T
                            # accumulator read by the Wo projection.
                            # l lands at partition base 0 in SBUF first:
                            # reciprocal_approx_fast misbehaves on sources at
                            # partition base 64 (SBUF or PSUM)
                            o_l = p2.tile([64, ST], f32, tag="ol", bufs=2)
                            nc.vector.tensor_copy(o_l, o_ps[64:128, :])
                            rl = p2.tile([64, ST], f32, tag="rl", bufs=2)
                            nc.vector.reciprocal_approx_fast(rl, o_l)
                            nc.vector.tensor_tensor(ot_acc[hb:hb + 64, j, :],
                                                    o_ps[0:64, :], rl,
                                                    OP.mult)
                            if debug_taps and qt == 0 and h == 0:
                                nc.sync.dma_start(dbg_of[64:128, :], o_l)
                                nc.sync.dma_start(dbg_rl, rl)
                                nc.sync.dma_start(dbg_qt, qt_tile)
                                nc.sync.dma_start(dbg_kt, KT[:, :, 0:ST])
                                nc.sync.dma_start(dbg_v, V[:, 0, :, :])

                    U, G = len(units), len(glist)
                    emitted = 0
                    for i in range(G):
                        if i >= 2:
                            emit_tail(i - 2)
                        emit_qk(i)
                        target = (i + 1) * U // G
                        while emitted < target:
                            units[emitted]()
                            emitted += 1
                    if G >= 2:
                        emit_tail(G - 2)
                    emit_tail(G - 1)
                    while emitted < U:
                        units[emitted]()
                        emitted += 1

                # startup: interleave x-tile 0 and weight chunk DMAs so the
                # first projection matmul (needs xt chunk 0 + wqkv chunk 0)
                # starts after ~2 chunks, not the full 5MB
                wqkv_sb = p1.tile([P, NIC, 768], f32r, tag="wqkv", bufs=1)
                wq3 = wqkv_d.rearrange("(a p) f -> p a f", p=P)
                xt0 = p1.tile([P, NIC, ST], f32r, tag="xt", bufs=2)
                for _ic in range(NIC):
                    nc.sync.dma_start(xt0[:, _ic, :], xT3[:, _ic, 0:ST])
                    nc.sync.dma_start(wqkv_sb[:, _ic, :], wq3[:, _ic, :])
                xt_tiles[0] = xt0
                cs0 = p1.tile([P, 2, ST], f32, tag="cs", bufs=2)
                nc.sync.dma_start(cs0[:, 0, :], cos_d[:, 0:ST])
                nc.sync.dma_start(cs0[:, 1, :], sin_d[:, 0:ST])
                cs_tiles[0] = cs0
                mask_sb = p2.tile([P, 4, ST], bf16, tag="mask", bufs=1)
                nc.sync.dma_start(mask_sb,
                                  mask_d.rearrange("p (c s) -> p c s", c=4))
                wo_sb = p1.tile([P, 2, D_MODEL], bf16, tag="wo", bufs=1)
                nc.sync.dma_start(wo_sb, wo_d.rearrange("(a p) f -> p a f", p=P))
                for it in range(NT + 2):
                    units = []
                    if it < NT:
                        units += make_proj_units(it)
                    if it >= 2:
                        units += make_wo_units(it - 2)
                    if 1 <= it <= NT:
                        emit_attention(it - 1, units)
                    else:
                        for u in units:
                            u()

    nc.compile()
    return nc


def _round_fp32r(a):
    """Round fp32 to the fp32r format (1s + 8e + 11m in the top 20 bits, RNE).

    The PE consumes float32r operands pre-rounded to 11 mantissa bits; doing
    the rounding on the host makes DMA-fed operands valid fp32r producers.
    """
    b = np.ascontiguousarray(a, dtype=np.float32).view(np.uint32)
    lsb = (b >> np.uint32(12)) & np.uint32(1)
    r = (b + np.uint32(0x7FF) + lsb) & np.uint32(0xFFFFF000)
    return r.view(np.float32)


def make_core_inputs(x, token_positions, Wq, Wk, Wv, Wo, S=SEQ):
    """Host-side sharding/layout prep. Returns in_maps for the 8 cores."""
    x = np.asarray(x, dtype=np.float32)
    Wq = np.asarray(Wq, dtype=np.float32)
    Wk = np.asarray(Wk, dtype=np.float32)
    Wv = np.asarray(Wv, dtype=np.float32)
    Wo = np.asarray(Wo, dtype=np.float32)
    pos = np.asarray(token_positions).astype(np.float32)

    scale = np.float32(1.0 / np.sqrt(np.float32(D_K)))
    half = D_K // 2
    inv_freq = (1.0 / (np.float32(THETA) **
                       (np.arange(0, D_K, 2, dtype=np.float32) / np.float32(D_K))
                       )).astype(np.float32)
    freqs = pos[:, None] * inv_freq[None, :]          # [S, 32]
    cosT = np.cos(freqs).T.astype(np.float32)         # [32, S]
    sinT = np.sin(freqs).T.astype(np.float32)
    cos4 = np.ascontiguousarray(np.tile(cosT, (HPC, 1)))  # [128, S]
    sin4 = np.ascontiguousarray(np.tile(sinT, (HPC, 1)))

    # diagonal-block causal masks: dmask[i, c, j] = 1 if j >= i + 128*c
    ii = np.arange(P)[:, None]
    jj = np.arange(ST)[None, :]
    dmask = np.stack([(jj >= ii + P * c).astype(np.float32) for c in range(4)],
                     axis=1).reshape(P, 4 * ST)
    dmask = np.ascontiguousarray(dmask.astype(ml_dtypes.bfloat16))

    xTs = [np.ascontiguousarray(x[b].T) for b in range(BATCH)]      # [D, S]

    in_maps = []
    for c in range(N_CORES):
        b, g = c // 4, c % 4
        # permutation: wq/wk output dims -> [all 4 heads' evens | all odds]
        perm = np.empty(256, dtype=np.int64)
        for t in range(HPC):
            hg = HPC * g + t
            perm[t * half:(t + 1) * half] = hg * D_K + 2 * np.arange(half)
            perm[128 + t * half:128 + (t + 1) * half] = \
                hg * D_K + 2 * np.arange(half) + 1
        wqT = (Wq[perm, :] * scale).T                               # [D, 256]
        wkT = Wk[perm, :].T                                         # [D, 256]
        wvT = Wv[g * 256:(g + 1) * 256, :].T                        # [D, 256]
        wqkvT = np.ascontiguousarray(
            np.concatenate([wqT, wkT, wvT], axis=1).astype(np.float32))
        woT = np.ascontiguousarray(
            Wo[:, g * 256:(g + 1) * 256].T.astype(ml_dtypes.bfloat16))
        in_maps.append({
            "xT": _round_fp32r(xTs[b]),
            "wqkvT": _round_fp32r(wqkvT),
            "woT": woT,
            "cos4": cos4,
            "sin4": sin4,
            "dmask": dmask,
        })
    return in_maps


_PROGRAM_CACHE = {}


def _get_program(S=SEQ):
    if S not in _PROGRAM_CACHE:
        _PROGRAM_CACHE[S] = build_program(S)
    return _PROGRAM_CACHE[S]


def run_cores(in_maps, trace=False, **kwargs):
    nc = _get_program(SEQ)
    return run_bass_kernel_spmd(nc, in_maps, core_ids=list(range(N_CORES)),
                                trace=trace, **kwargs)


def kernel(x, token_positions, Wq, Wk, Wv, Wo):
    in_maps = make_core_inputs(x, token_positions, Wq, Wk, Wv, Wo)
    res = run_cores(in_maps)
    out = np.zeros((BATCH, SEQ, D_MODEL), dtype=np.float32)
    for c in range(N_CORES):
        out[c // 4] += res.results[c]["y"]
    return out


_TIMED_CACHE = {}


def run_cores_timed(in_maps, iters=8, program=None):
    """Execute the SPMD program with device-resident inputs repeatedly and
    return (per-exec wall seconds list, outputs-per-core). Used for timing
    only — the NTFF profiling hook is unavailable under this axon client."""
    import time

    import jax
    from jax.experimental.shard_map import shard_map
    from jax.sharding import Mesh, NamedSharding, PartitionSpec

    from concourse.bass2jax import (
        _bass_exec_p,
        install_neuronx_cc_hook,
        partition_id_tensor,
    )

    nc = program if program is not None else _get_program(SEQ)

    if id(nc) in _TIMED_CACHE:
        sharded, dev_in, out_avals, out_names, n_cores = _TIMED_CACHE[id(nc)]
        out = sharded(*dev_in)
        jax.block_until_ready(out)
        times = []
        for _ in range(iters):
            t0 = time.perf_counter()
            out = sharded(*dev_in)
            jax.block_until_ready(out)
            times.append(time.perf_counter() - t0)
        results = [
            {name: np.asarray(out[i]).reshape(n_cores, *out_avals[i].shape)[c]
             for i, name in enumerate(out_names)}
            for c in range(n_cores)
        ]
        return times, results
    install_neuronx_cc_hook()
    partition_name = nc.partition_id_tensor.name if nc.partition_id_tensor else None
    in_names, out_names, out_avals, zero_outs = [], [], [], []
    for alloc in nc.m.functions[0].allocations:
        if not isinstance(alloc, mybir.MemoryLocationSet):
            continue
        name = alloc.memorylocations[0].name
        if alloc.kind == "ExternalInput":
            if name != partition_name:
                in_names.append(name)
        elif alloc.kind == "ExternalOutput":
            out_names.append(name)
            shape = tuple(alloc.tensor_shape)
            dtype = mybir.dt.np(alloc.dtype)
            out_avals.append(jax.core.ShapedArray(shape, dtype))
            zero_outs.append(np.zeros(shape, dtype))
    n_params = len(in_names)
    all_names = in_names + out_names + ([partition_name] if partition_name else [])

    def _body(*args):
        operands = list(args)
        if partition_name:
            operands.append(partition_id_tensor())
        outs = _bass_exec_p.bind(
            *operands,
            out_avals=tuple(out_avals),
            in_names=tuple(all_names),
            out_names=tuple(out_names),
            lowering_input_output_aliases=(),
            sim_require_finite=True,
            sim_require_nnan=True,
            nc=nc,
        )
        return tuple(outs)

    n_cores = len(in_maps)
    devices = jax.devices()[:n_cores]
    mesh = Mesh(np.asarray(devices), ("core",))
    nin = n_params + len(out_names)
    sharded = jax.jit(
        shard_map(_body, mesh=mesh,
                  in_specs=(PartitionSpec("core"),) * nin,
                  out_specs=(PartitionSpec("core"),) * len(out_names),
                  check_rep=False),
        keep_unused=True)
    per_core = [[np.asarray(m[n]) for n in in_names] for m in in_maps]
    concat_in = [np.concatenate([per_core[c][i] for c in range(n_cores)], axis=0)
                 for i in range(n_params)]
    concat_zeros = [np.zeros((n_cores * z.shape[0], *z.shape[1:]), z.dtype)
                    for z in zero_outs]
    sh = NamedSharding(mesh, PartitionSpec("core"))
    dev_in = [jax.device_put(a, sh) for a in concat_in + concat_zeros]
    _TIMED_CACHE[id(nc)] = (sharded, dev_in, out_avals, out_names, n_cores)
    out = sharded(*dev_in)
    jax.block_until_ready(out)
    times = []
    for _ in range(iters):
        t0 = time.perf_counter()
        out = sharded(*dev_in)
        jax.block_until_ready(out)
        times.append(time.perf_counter() - t0)
    results = [
        {name: np.asarray(out[i]).reshape(n_cores, *out_avals[i].shape)[c]
         for i, name in enumerate(out_names)}
        for c in range(n_cores)
    ]
    return times, results



# revision 8
# speedup vs baseline: 1.1008x; 1.1008x over previous
"""Causal multi-head self-attention (B=2, S=4096, D=1024, H=16, dk=64) on 8 trn2 cores.

Sharding: core c handles batch b = c // 4 and heads [4*(c%4) .. 4*(c%4)+3]
(data parallel on B, tensor parallel on heads / QKV / O projections).
Each core returns a partial [S, D] output (its heads' contribution after the
Wo projection); the host sums the 4 partials per batch.

Device-side design (per core):
  - host supplies xT = x[b].T so every projection contracts over the model dim
    on partitions; Wq/Wk columns are host-permuted so RoPE is rotate-half form
    (full-width DVE ops), then SBUF->SBUF DMAs re-group rotated rows
    head-contiguously (bf16) for the K=64 QK^T contraction.
  - attention runs in S^T layout (scores [k, q]). V carries 64 ones-columns
    per head, so each PV matmul emits O^T on partitions 0:64 AND the softmax
    denominator l broadcast across partitions 64:128 - normalization is just
    copy + reciprocal + multiply, no partition-axis reduction or shuffle.
  - K/Q/V/probs/Wo run in bf16 (same PE rate, half SBUF, 2x DVE on masks);
    projections contract in float32r, PSUM accumulates fp32 throughout.
  - instruction-level software pipeline: attention for q-tile t-1 is the
    backbone (QK staging runs 2 key-block groups ahead of exp -> mask -> PV),
    and the projection / Wo-output matmul chunks of neighboring tiles are
    spread between groups so the PE never drains while ACT computes exp.
"""

import numpy as np
import ml_dtypes

import concourse.bass as bass
import concourse.bacc as bacc
import concourse.mybir as mybir
import concourse.tile as tile
from concourse.bass_utils import run_bass_kernel_spmd

P = 128
D_MODEL = 1024
N_HEADS = 16
D_K = 64
SEQ = 4096
BATCH = 2
N_CORES = 8
HPC = 4  # heads per core
ST = 512  # s-tile / q-tile width
GK = 2  # key-blocks per staging / exp group
THETA = 10000.0

f32 = mybir.dt.float32
f32r = mybir.dt.float32r
bf16 = mybir.dt.bfloat16
i16 = mybir.dt.int16
AF = mybir.ActivationFunctionType
OP = mybir.AluOpType

# Schraudolph fast-exp constants for bf16 bit patterns computed as int16:
# bits = round_to_nearest(x * 128/ln2 + B); reinterpret int16 as bf16.
# B tuned on-host for min RMS rel err (~1.8%, max ~4%) given the DVE's
# round-to-nearest float->int16 conversion (verified on hw). A constant
# multiplicative bias in exp cancels between softmax numerator/denominator.
EXP_A = float(np.float32(128.0 / np.log(2.0)))
EXP_B = 16248.5
# fraction of exp groups routed to DVE fast-exp instead of ACT exact exp:
# indices c with (c % EXP_MOD) in EXP_DVE_RES go to DVE
EXP_MOD = 7
EXP_DVE_RES = (2, 5)


def build_program(S=SEQ, reps=1, debug_taps=False):
    nc = bacc.Bacc("TRN2", target_bir_lowering=False, debug=False,
                   num_devices=N_CORES)

    NT = S // ST          # number of s-tiles == number of q-tiles
    NKB = S // P          # number of 128-wide key blocks
    NIC = D_MODEL // P    # contraction chunks over the model dim

    xT_d = nc.dram_tensor("xT", [D_MODEL, S], f32r, kind="ExternalInput").ap()
    wqkv_d = nc.dram_tensor("wqkvT", [D_MODEL, 768], f32r, kind="ExternalInput").ap()
    wo_d = nc.dram_tensor("woT", [256, D_MODEL], bf16, kind="ExternalInput").ap()
    cos_d = nc.dram_tensor("cos4", [P, S], f32, kind="ExternalInput").ap()
    sin_d = nc.dram_tensor("sin4", [P, S], f32, kind="ExternalInput").ap()
    mask_d = nc.dram_tensor("dmask", [P, 4 * ST], bf16, kind="ExternalInput").ap()
    y_d = nc.dram_tensor("y", [S, D_MODEL], f32, kind="ExternalOutput").ap()
    if debug_taps:
        dbg_es = nc.dram_tensor("dbg_es", [P, GK, ST], bf16,
                                kind="ExternalOutput").ap()
        dbg_of = nc.dram_tensor("dbg_of", [P, ST], f32,
                                kind="ExternalOutput").ap()
        dbg_rl = nc.dram_tensor("dbg_rl", [64, ST], f32,
                                kind="ExternalOutput").ap()
        dbg_qt = nc.dram_tensor("dbg_qt", [P, 2, ST], bf16,
                                kind="ExternalOutput").ap()
        dbg_kt = nc.dram_tensor("dbg_kt", [P, 2, ST], bf16,
                                kind="ExternalOutput").ap()
        dbg_v = nc.dram_tensor("dbg_v", [P, HPC, 128], bf16,
                               kind="ExternalOutput").ap()

    with tile.TileContext(nc) as tc:
      for _rep in range(reps):
        with tc.tile_pool(name="res", bufs=1) as res:
            KT = res.tile([P, 2, S], bf16)       # rotated K^T, head-contiguous
            # 64 ones + 64 value cols per head: ones FIRST so the softmax
            # denominator lands at PSUM partitions 0:64, where
            # reciprocal_approx_fast can read it directly (no staging copy)
            V = res.tile([P, NKB, HPC, 128], bf16)
            nc.gpsimd.memset(V[:, :, :, 0:64], 1.0)

            with tc.tile_pool(name="p1", bufs=2) as p1, \
                 tc.tile_pool(name="rot", bufs=2) as rotp, \
                 tc.tile_pool(name="p2", bufs=2) as p2, \
                 tc.tile_pool(name="ppp", bufs=2, space="PSUM") as ppp, \
                 tc.tile_pool(name="stg", bufs=2, space="PSUM") as stgp, \
                 tc.tile_pool(name="opp", bufs=2, space="PSUM") as opp:
                xT3 = xT_d.rearrange("(a p) s -> p a s", p=P)
                xt_tiles, cs_tiles, qt_tiles, ot_tiles = {}, {}, {}, {}
                exp_cnt = [0]  # round-robin index for the ACT/DVE exp split

                def dma_in(t):
                    # per-chunk DMAs so the first projection matmul only
                    # waits on chunk 0, not the whole 2MB tile
                    tsl = slice(t * ST, (t + 1) * ST)
                    xt = p1.tile([P, NIC, ST], f32r, tag="xt", bufs=2)
                    for ic in range(NIC):
                        nc.sync.dma_start(xt[:, ic, :], xT3[:, ic, tsl])
                    cs = p1.tile([P, 2, ST], f32, tag="cs", bufs=2)
                    nc.sync.dma_start(cs[:, 0, :], cos_d[:, tsl])
                    nc.sync.dma_start(cs[:, 1, :], sin_d[:, tsl])
                    xt_tiles[t] = xt
                    cs_tiles[t] = cs

                def make_proj_units(t):
                    """PE/DVE/DMA work units for projecting s-tile t."""
                    tsl = slice(t * ST, (t + 1) * ST)
                    state = {}

                    def mm_qk(col0, key, pool, ptag):
                        def u():
                            xt = xt_tiles[t]
                            pe_ps = pool.tile([P, ST], f32, tag=ptag)
                            po_ps = pool.tile([P, ST], f32, tag=ptag)
                            for ic in range(NIC):
                                nc.tensor.matmul(
                                    pe_ps,
                                    lhsT=wqkv_sb[:, ic, col0:col0 + P],
                                    rhs=xt[:, ic, :],
                                    start=(ic == 0), stop=(ic == NIC - 1))
                                nc.tensor.matmul(
                                    po_ps,
                                    lhsT=wqkv_sb[:, ic, col0 + P:col0 + 256],
                                    rhs=xt[:, ic, :],
                                    start=(ic == 0), stop=(ic == NIC - 1))
                            state[key] = (pe_ps, po_ps)
                        return u

                    def rope(key, is_q):
                        def u():
                            pe_ps, po_ps = state.pop(key)
                            cs = cs_tiles[t]
                            if is_q:
                                dst = p2.tile([P, 2, ST], bf16, tag="qt",
                                              bufs=2)
                                qt_tiles[t] = dst
                                dsl = slice(0, ST)
                            else:
                                dst = KT
                                dsl = tsl
                            t1 = rotp.tile([P, ST], f32, tag="tmp", bufs=5)
                            t3 = rotp.tile([P, ST], f32, tag="tmp", bufs=5)
                            nc.vector.tensor_tensor(t1, pe_ps, cs[:, 0, :], OP.mult)
                            nc.vector.tensor_tensor(t3, pe_ps, cs[:, 1, :], OP.mult)
                            t2 = rotp.tile([P, ST], f32, tag="tmp", bufs=5)
                            t4 = rotp.tile([P, ST], f32, tag="tmp", bufs=5)
                            nc.vector.tensor_tensor(t2, po_ps, cs[:, 1, :], OP.mult)
                            nc.vector.tensor_tensor(t4, po_ps, cs[:, 0, :], OP.mult)
                            rot_e = rotp.tile([P, ST], bf16, tag="re")
                            rot_o = rotp.tile([P, ST], bf16, tag="ro")
                            # SBUF-only combines go to the otherwise-idle Pool
                            # engine (gpsimd has no PSUM port, so the mults
                            # above, which read PSUM, must stay on DVE)
                            nc.gpsimd.tensor_tensor(rot_e, t1, t2, OP.subtract)
                            nc.gpsimd.tensor_tensor(rot_o, t3, t4, OP.add)
                            # scatter rotated rows head-contiguously:
                            # head h evens -> dst[(h%2)*64 +  0 .. +32, h//2]
                            # head h odds  -> dst[(h%2)*64 + 32 .. +64, h//2]
                            for j in (0, 1):
                                for hh in (0, 1):
                                    h = 2 * j + hh
                                    nc.sync.dma_start(
                                        dst[hh * 64:hh * 64 + 32, j, dsl],
                                        rot_e[h * 32:(h + 1) * 32, :])
                                    nc.sync.dma_start(
                                        dst[hh * 64 + 32:hh * 64 + 64, j, dsl],
                                        rot_o[h * 32:(h + 1) * 32, :])
                        return u

                    def vproj(sb0):
                        def u():
                            xt = xt_tiles[t]
                            for sb in (sb0, sb0 + 1):
                                kb = t * (ST // P) + sb
                                v_ps = ppp.tile([P, 256], f32, tag="pp")
                                for ic in range(NIC):
                                    nc.tensor.matmul(
                                        v_ps,
                                        lhsT=xt[:, ic, sb * P:(sb + 1) * P],
                                        rhs=wqkv_sb[:, ic, 512:768],
                                        start=(ic == 0), stop=(ic == NIC - 1))
                                nc.vector.tensor_copy(
                                    V[:, kb, :, 64:128],
                                    v_ps.rearrange("p (h c) -> p h c", h=HPC))
                        return u

                    units = []
                    if t + 1 < NT:
                        units.append(lambda: dma_in(t + 1))
                    # tile 0 runs before any attention exists to hide PSUM
                    # write-after-read stalls; spread it over the idle
                    # attention pools for extra bank parallelism
                    if t == 0:
                        units.append(mm_qk(0, "q", ppp, "pp"))
                        units.append(mm_qk(256, "k", opp, "o"))
                        units.append(rope("q", True))
                        units.append(rope("k", False))
                    else:
                        units.append(mm_qk(0, "q", ppp, "pp"))
                        units.append(rope("q", True))
                        units.append(mm_qk(256, "k", ppp, "pp"))
                        units.append(rope("k", False))
                    units.append(vproj(0))
                    units.append(vproj(2))
                    return units

                def make_wo_units(t):
                    """Wo projection for q-tile t's 4 output row-blocks."""
                    ot = ot_tiles.pop(t)
                    state = {}

                    def u_make(sb, nh):
                        def u():
                            if nh == 0:
                                state[sb] = p2.tile([P, D_MODEL], f32,
                                                    tag="y", bufs=2,
                                                    name=f"out_t{t}_{sb}")
                            out_t = state[sb]
                            y_ps = ppp.tile([P, 512], f32, tag="pp")
                            for j in (0, 1):
                                nc.tensor.matmul(
                                    y_ps,
                                    lhsT=ot[:, j, sb * P:(sb + 1) * P],
                                    rhs=wo_sb[:, j, nh * 512:(nh + 1) * 512],
                                    start=(j == 0), stop=(j == 1))
                            nc.vector.tensor_copy(
                                out_t[:, nh * 512:(nh + 1) * 512], y_ps)
                            if nh == 1:
                                nc.sync.dma_start(
                                    y_d[t * ST + sb * P:t * ST + (sb + 1) * P, :],
                                    out_t)
                        return u

                    return [u_make(sb, nh) for sb in range(ST // P)
                            for nh in (0, 1)]

                def emit_attention(qt, units):
                    """Attention for q-tile qt with a 2-group run-ahead;
                    `units` (proj/wo chunks) are spread between groups."""
                    nkb = (qt + 1) * (ST // P)
                    qt_tile = qt_tiles.pop(qt)
                    ot_acc = p2.tile([P, 2, ST], bf16, tag="ota", bufs=2)
                    ot_tiles[qt] = ot_acc

                    # head pairs (even, odd) are interleaved group-by-group:
                    # even heads sit at PE base partition 0, odd heads at 64,
                    # so adjacent K=64 matmuls land on disjoint row-groups of
                    # the PE array and run concurrently (~2x QK throughput)
                    glist = []  # (h, g0, glen, q0s, is_last_of_head)
                    for hp in range(HPC // 2):
                        for g0 in range(0, nkb, GK):
                            glen = min(GK, nkb - g0)
                            q0s = 256 if g0 - qt * (ST // P) >= 2 else 0
                            last = g0 + GK >= nkb
                            for h in (2 * hp, 2 * hp + 1):
                                glist.append((h, g0, glen, q0s, last))
                    stgs = [None] * len(glist)
                    o_ps_by_head = {}

                    def emit_qk(i):
                        h, g0, glen, q0s, _ = glist[i]
                        j, hb = h // 2, (h % 2) * 64
                        stg = stgp.tile([P, GK, ST], f32, tag="s")
                        for gi in range(glen):
                            kb = g0 + gi
                            # per-block trim: cols < 128*c are fully masked
                            # and never consumed downstream (exp output in
                            # [q0s:q0k) is discarded garbage)
                            c = kb - qt * (ST // P)
                            q0k = max(c, 0) * P
                            nc.tensor.matmul(
                                stg[:, gi, q0k:],
                                lhsT=KT[hb:hb + 64, j, kb * P:(kb + 1) * P],
                                rhs=qt_tile[hb:hb + 64, j, q0k:],
                                start=True, stop=True)
                        stgs[i] = stg

                    def emit_tail(i):
                        h, g0, glen, q0s, last = glist[i]
                        j, hb = h // 2, (h % 2) * 64
                        stg = stgs[i]
                        stgs[i] = None
                        es = p2.tile([P, GK, ST], bf16, tag="e", bufs=3)
                        # exp is split between ACT (exact spline exp) and DVE
                        # (Schraudolph int16 bit-trick, ~1.8% rms) so the ACT
                        # engine's 1 elem/cycle/lane ceiling stops being the
                        # pipeline's rate limiter
                        ci = exp_cnt[0]
                        exp_cnt[0] += 1
                        if ci % EXP_MOD in EXP_DVE_RES:
                            es16 = es.bitcast(i16)
                            nc.vector.tensor_scalar(
                                es16[:, :glen, q0s:], stg[:, :glen, q0s:],
                                EXP_A, EXP_B, OP.mult, OP.add)
                        else:
                            nc.scalar.activation(es[:, :glen, q0s:],
                                                 stg[:, :glen, q0s:], AF.Exp)
                        if g0 == 0:
                            o_ps_by_head[h] = opp.tile(
                                [P, ST], f32, tag="o", name=f"o_ps_h{h}")
                        o_ps = o_ps_by_head[h]
                        for gi in range(glen):
                            kb = g0 + gi
                            c = kb - qt * (ST // P)
                            q0p = max(c, 0) * P
                            if c >= 0:
                                # zero exp(S^T)[i, jq] where jq < i + 128*c
                                # (SBUF-only op -> Pool engine, DVE is loaded)
                                nc.gpsimd.tensor_tensor(
                                    es[:, gi, q0p:], es[:, gi, q0p:],
                                    mask_sb[:, c, q0p:], OP.mult)
                            nc.tensor.matmul(
                                o_ps[:, q0p:],
                                lhsT=V[:, kb, h, :],
                                rhs=es[:, gi, q0p:],
                                start=(kb == 0), stop=(kb == nkb - 1))
                        if debug_taps and qt == 0 and h == 0 and g0 == 0:
                            nc.sync.dma_start(dbg_es, es)
                        if last:
                            # rows 0:64 of o_ps hold l broadcast by the
                            # ones-columns of V (ones first); the fast
                            # reciprocal reads it straight from PSUM at
                            # partition base 0, values sit at 64:128
                            rl = p2.tile([64, ST], f32, tag="rl", bufs=2)
                            nc.vector.reciprocal_approx_fast(rl, o_ps[0:64, :])
                            nc.vector.tensor_tensor(ot_acc[hb:hb + 64, j, :],
                                                    o_ps[64:128, :], rl,
                                                    OP.mult)
                            if debug_taps and qt == 0 and h == 0:
                                nc.sync.dma_start(dbg_of[0:64, :],
                                                  o_ps[0:64, :])
                                nc.sync.dma_start(dbg_rl, rl)
                                nc.sync.dma_start(dbg_qt, qt_tile)
                                nc.sync.dma_start(dbg_kt, KT[:, :, 0:ST])
                                nc.sync.dma_start(dbg_v, V[:, 0, :, :])

                    U, G = len(units), len(glist)
                    # pair-atomic emission: the two QK chains of an
                    # (even, odd) head pair are emitted back-to-back with
                    # nothing between them, so their K=64 matmuls overlap on
                    # disjoint PE row-groups; tails and proj/wo units go
                    # between pairs, one pair of run-ahead
                    emitted = 0
                    NPAIR = G // 2
                    for ip in range(NPAIR):
                        if ip >= 1:
                            emit_tail(2 * ip - 2)
                        emit_qk(2 * ip)
                        emit_qk(2 * ip + 1)
                        if ip >= 1:
                            emit_tail(2 * ip - 1)
                        target = (ip + 1) * U // NPAIR
                        while emitted < target:
                            units[emitted]()
                            emitted += 1
                    emit_tail(G - 2)
                    emit_tail(G - 1)
                    while emitted < U:
                        units[emitted]()
                        emitted += 1

                # startup: interleave x-tile 0 and weight chunk DMAs so the
                # first projection matmul (needs xt chunk 0 + wqkv chunk 0)
                # starts after ~2 chunks, not the full 5MB
                wqkv_sb = p1.tile([P, NIC, 768], f32r, tag="wqkv", bufs=1)
                wq3 = wqkv_d.rearrange("(a p) f -> p a f", p=P)
                xt0 = p1.tile([P, NIC, ST], f32r, tag="xt", bufs=2)
                for _ic in range(NIC):
                    nc.sync.dma_start(xt0[:, _ic, :], xT3[:, _ic, 0:ST])
                    nc.sync.dma_start(wqkv_sb[:, _ic, :], wq3[:, _ic, :])
                xt_tiles[0] = xt0
                cs0 = p1.tile([P, 2, ST], f32, tag="cs", bufs=2)
                nc.sync.dma_start(cs0[:, 0, :], cos_d[:, 0:ST])
                nc.sync.dma_start(cs0[:, 1, :], sin_d[:, 0:ST])
                cs_tiles[0] = cs0
                mask_sb = p2.tile([P, 4, ST], bf16, tag="mask", bufs=1)
                nc.sync.dma_start(mask_sb,
                                  mask_d.rearrange("p (c s) -> p c s", c=4))
                wo_sb = p1.tile([P, 2, D_MODEL], bf16, tag="wo", bufs=1)
                nc.sync.dma_start(wo_sb, wo_d.rearrange("(a p) f -> p a f", p=P))
                for it in range(NT + 2):
                    units = []
                    if it < NT:
                        units += make_proj_units(it)
                    if it >= 2:
                        units += make_wo_units(it - 2)
                    if 1 <= it <= NT:
                        emit_attention(it - 1, units)
                    else:
                        for u in units:
                            u()

    nc.compile()
    return nc


def _round_fp32r(a):
    """Round fp32 to the fp32r format (1s + 8e + 11m in the top 20 bits, RNE).

    The PE consumes float32r operands pre-rounded to 11 mantissa bits; doing
    the rounding on the host makes DMA-fed operands valid fp32r producers.
    """
    b = np.ascontiguousarray(a, dtype=np.float32).view(np.uint32)
    lsb = (b >> np.uint32(12)) & np.uint32(1)
    r = (b + np.uint32(0x7FF) + lsb) & np.uint32(0xFFFFF000)
    return r.view(np.float32)


def make_core_inputs(x, token_positions, Wq, Wk, Wv, Wo, S=SEQ):
    """Host-side sharding/layout prep. Returns in_maps for the 8 cores."""
    x = np.asarray(x, dtype=np.float32)
    Wq = np.asarray(Wq, dtype=np.float32)
    Wk = np.asarray(Wk, dtype=np.float32)
    Wv = np.asarray(Wv, dtype=np.float32)
    Wo = np.asarray(Wo, dtype=np.float32)
    pos = np.asarray(token_positions).astype(np.float32)

    scale = np.float32(1.0 / np.sqrt(np.float32(D_K)))
    half = D_K // 2
    inv_freq = (1.0 / (np.float32(THETA) **
                       (np.arange(0, D_K, 2, dtype=np.float32) / np.float32(D_K))
                       )).astype(np.float32)
    freqs = pos[:, None] * inv_freq[None, :]          # [S, 32]
    cosT = np.cos(freqs).T.astype(np.float32)         # [32, S]
    sinT = np.sin(freqs).T.astype(np.float32)
    cos4 = np.ascontiguousarray(np.tile(cosT, (HPC, 1)))  # [128, S]
    sin4 = np.ascontiguousarray(np.tile(sinT, (HPC, 1)))

    # diagonal-block causal masks: dmask[i, c, j] = 1 if j >= i + 128*c
    ii = np.arange(P)[:, None]
    jj = np.arange(ST)[None, :]
    dmask = np.stack([(jj >= ii + P * c).astype(np.float32) for c in range(4)],
                     axis=1).reshape(P, 4 * ST)
    dmask = np.ascontiguousarray(dmask.astype(ml_dtypes.bfloat16))

    xTs = [np.ascontiguousarray(x[b].T) for b in range(BATCH)]      # [D, S]

    in_maps = []
    for c in range(N_CORES):
        b, g = c // 4, c % 4
        # permutation: wq/wk output dims -> [all 4 heads' evens | all odds]
        perm = np.empty(256, dtype=np.int64)
        for t in range(HPC):
            hg = HPC * g + t
            perm[t * half:(t + 1) * half] = hg * D_K + 2 * np.arange(half)
            perm[128 + t * half:128 + (t + 1) * half] = \
                hg * D_K + 2 * np.arange(half) + 1
        wqT = (Wq[perm, :] * scale).T                               # [D, 256]
        wkT = Wk[perm, :].T                                         # [D, 256]
        wvT = Wv[g * 256:(g + 1) * 256, :].T                        # [D, 256]
        wqkvT = np.ascontiguousarray(
            np.concatenate([wqT, wkT, wvT], axis=1).astype(np.float32))
        woT = np.ascontiguousarray(
            Wo[:, g * 256:(g + 1) * 256].T.astype(ml_dtypes.bfloat16))
        in_maps.append({
            "xT": _round_fp32r(xTs[b]),
            "wqkvT": _round_fp32r(wqkvT),
            "woT": woT,
            "cos4": cos4,
            "sin4": sin4,
            "dmask": dmask,
        })
    return in_maps


_PROGRAM_CACHE = {}


def _get_program(S=SEQ):
    if S not in _PROGRAM_CACHE:
        _PROGRAM_CACHE[S] = build_program(S)
    return _PROGRAM_CACHE[S]


def run_cores(in_maps, trace=False, **kwargs):
    nc = _get_program(SEQ)
    return run_bass_kernel_spmd(nc, in_maps, core_ids=list(range(N_CORES)),
                                trace=trace, **kwargs)


def kernel(x, token_positions, Wq, Wk, Wv, Wo):
    in_maps = make_core_inputs(x, token_positions, Wq, Wk, Wv, Wo)
    res = run_cores(in_maps)
    out = np.zeros((BATCH, SEQ, D_MODEL), dtype=np.float32)
    for c in range(N_CORES):
        out[c // 4] += res.results[c]["y"]
    return out


_TIMED_CACHE = {}


def run_cores_timed(in_maps, iters=8, program=None):
    """Execute the SPMD program with device-resident inputs repeatedly and
    return (per-exec wall seconds list, outputs-per-core). Used for timing
    only — the NTFF profiling hook is unavailable under this axon client."""
    import time

    import jax
    from jax.experimental.shard_map import shard_map
    from jax.sharding import Mesh, NamedSharding, PartitionSpec

    from concourse.bass2jax import (
        _bass_exec_p,
        install_neuronx_cc_hook,
        partition_id_tensor,
    )

    nc = program if program is not None else _get_program(SEQ)

    if id(nc) in _TIMED_CACHE:
        sharded, dev_in, out_avals, out_names, n_cores = _TIMED_CACHE[id(nc)]
        out = sharded(*dev_in)
        jax.block_until_ready(out)
        times = []
        for _ in range(iters):
            t0 = time.perf_counter()
            out = sharded(*dev_in)
            jax.block_until_ready(out)
            times.append(time.perf_counter() - t0)
        results = [
            {name: np.asarray(out[i]).reshape(n_cores, *out_avals[i].shape)[c]
             for i, name in enumerate(out_names)}
            for c in range(n_cores)
        ]
        return times, results
    install_neuronx_cc_hook()
    partition_name = nc.partition_id_tensor.name if nc.partition_id_tensor else None
    in_names, out_names, out_avals, zero_outs = [], [], [], []
    for alloc in nc.m.functions[0].allocations:
        if not isinstance(alloc, mybir.MemoryLocationSet):
            continue
        name = alloc.memorylocations[0].name
        if alloc.kind == "ExternalInput":
            if name != partition_name:
                in_names.append(name)
        elif alloc.kind == "ExternalOutput":
            out_names.append(name)
            shape = tuple(alloc.tensor_shape)
            dtype = mybir.dt.np(alloc.dtype)
            out_avals.append(jax.core.ShapedArray(shape, dtype))
            zero_outs.append(np.zeros(shape, dtype))
    n_params = len(in_names)
    all_names = in_names + out_names + ([partition_name] if partition_name else [])

    def _body(*args):
        operands = list(args)
        if partition_name:
            operands.append(partition_id_tensor())
        outs = _bass_exec_p.bind(
            *operands,
            out_avals=tuple(out_avals),
            in_names=tuple(all_names),
            out_names=tuple(out_names),
            lowering_input_output_aliases=(),
            sim_require_finite=True,
            sim_require_nnan=True,
            nc=nc,
        )
        return tuple(outs)

    n_cores = len(in_maps)
    devices = jax.devices()[:n_cores]
    mesh = Mesh(np.asarray(devices), ("core",))
    nin = n_params + len(out_names)
    sharded = jax.jit(
        shard_map(_body, mesh=mesh,
                  in_specs=(PartitionSpec("core"),) * nin,
                  out_specs=(PartitionSpec("core"),) * len(out_names),
                  check_rep=False),
        keep_unused=True)
    per_core = [[np.asarray(m[n]) for n in in_names] for m in in_maps]
    concat_in = [np.concatenate([per_core[c][i] for c in range(n_cores)], axis=0)
                 for i in range(n_params)]
    concat_zeros = [np.zeros((n_cores * z.shape[0], *z.shape[1:]), z.dtype)
                    for z in zero_outs]
    sh = NamedSharding(mesh, PartitionSpec("core"))
    dev_in = [jax.device_put(a, sh) for a in concat_in + concat_zeros]
    _TIMED_CACHE[id(nc)] = (sharded, dev_in, out_avals, out_names, n_cores)
    out = sharded(*dev_in)
    jax.block_until_ready(out)
    times = []
    for _ in range(iters):
        t0 = time.perf_counter()
        out = sharded(*dev_in)
        jax.block_until_ready(out)
        times.append(time.perf_counter() - t0)
    results = [
        {name: np.asarray(out[i]).reshape(n_cores, *out_avals[i].shape)[c]
         for i, name in enumerate(out_names)}
        for c in range(n_cores)
    ]
    return times, results

